# revision 1
# baseline (speedup 1.0000x reference)
"""Trainium2 Bass kernel for nn_EncoderLayer (D=1024, H=16, S=2048, FF=4096), 8-core SPMD.

Strategy: head-parallel attention (2 heads/core), one 1MB AllToAll to switch to
sequence-parallel (256 positions/core) for the output projection, norms and FFN.
No all-reduce needed anywhere. v2: instruction/DMA-count minimized.
"""
import math
import os

import numpy as np

import concourse.bass as bass
import concourse.mybir as mybir
import concourse.tile as tile
from concourse import bacc
from concourse.bass_utils import run_bass_kernel_spmd
from concourse.masks import make_identity

F32 = mybir.dt.float32
AF = mybir.ActivationFunctionType

D = 1024
H = 16
HD = 64
S = 2048
FF = 4096
EPS = 1e-3
NCORES = 8
SL = S // NCORES          # 256 sequence positions per core after A2A
HPC = H // NCORES         # 2 heads per core
KT = D // 128             # 8 k-tiles over the model dim
TT = S // 128             # 16 t-tiles over sequence
SCH = 512                 # matmul moving-operand chunk (fp32 max)
NSCH = S // SCH           # 4 s-chunks
FFT = FF // 128           # 32 hidden tiles
UNBIAS = float(D) / float(D - 1)  # torch std uses ddof=1
ISCALE = 1.0 / math.sqrt(HD)


def _ln(nc, pools, x_sb, z_sb, a2_sb, b2n_sb, tag):
    """LayerNorm over free axis (1024) of x_sb [128, 1024] -> z_sb [128, 1024].

    Matches reference: (x - mu) / (std_ddof1 + eps) * a2 + b2.
    """
    s1 = pools.tile([128, 1], F32, tag=f"ln_s1", name=f"ln_s1_{tag}")
    nc.vector.reduce_sum(out=s1[:], in_=x_sb[:], axis=mybir.AxisListType.X)
    mu = pools.tile([128, 1], F32, tag=f"ln_mu", name=f"ln_mu_{tag}")
    nc.scalar.mul(mu[:], s1[:], 1.0 / D)
    xc = pools.tile([128, D], F32, tag=f"ln_xc", name=f"ln_xc_{tag}")
    nc.vector.tensor_scalar(out=xc[:], in0=x_sb[:], scalar1=mu[:], scalar2=None,
                            op0=mybir.AluOpType.subtract)
    sq = pools.tile([128, D], F32, tag=f"ln_sq", name=f"ln_sq_{tag}")
    nc.vector.tensor_mul(out=sq[:], in0=xc[:], in1=xc[:])
    s2 = pools.tile([128, 1], F32, tag=f"ln_s2", name=f"ln_s2_{tag}")
    nc.vector.reduce_sum(out=s2[:], in_=sq[:], axis=mybir.AxisListType.X)
    sig = pools.tile([128, 1], F32, tag=f"ln_sig", name=f"ln_sig_{tag}")
    # sigma = sqrt(ssq / (D-1)); then += eps; then reciprocal
    nc.scalar.activation(sig[:], s2[:], AF.Sqrt, scale=1.0 / (D - 1))
    nc.vector.tensor_scalar_add(sig[:], sig[:], EPS)
    rec = pools.tile([128, 1], F32, tag=f"ln_rec", name=f"ln_rec_{tag}")
    nc.vector.reciprocal(rec[:], sig[:])
    nc.vector.tensor_scalar_mul(z_sb[:], xc[:], rec[:])
    nc.vector.tensor_mul(out=z_sb[:], in0=z_sb[:], in1=a2_sb[:])
    nc.vector.tensor_add(out=z_sb[:], in0=z_sb[:], in1=b2n_sb[:])


def build(reps: int = 1):
    nc = bacc.Bacc("TRN2", target_bir_lowering=False, debug=False, num_devices=NCORES)

    # ---- DRAM parameters (per-core shards prepared on host) ----
    Qt = nc.declare_dram_parameter("Qt", [KT, 128, S], F32, isOutput=False)
    Kt = nc.declare_dram_parameter("Kt", [KT, 128, S], F32, isOutput=False)
    Vt = nc.declare_dram_parameter("Vt", [KT, 128, S], F32, isOutput=False)
    wqT = nc.declare_dram_parameter("wqT", [128, KT, 128], F32, isOutput=False)
    wkT = nc.declare_dram_parameter("wkT", [128, KT, 128], F32, isOutput=False)
    wvT = nc.declare_dram_parameter("wvT", [128, KT, 128], F32, isOutput=False)
    Wot = nc.declare_dram_parameter("Wot", [128, KT, D], F32, isOutput=False)
    # W1g[g] : [128, 4, 8, 128]  (ki, m-sub, kt, m)  contiguous 2MB blocks
    W1g = nc.declare_dram_parameter("W1g", [8, 128, 4, KT, 128], F32, isOutput=False)
    # W2g[g] : [128, 4, 1024]  (ki(f), kt-sub(f), d) contiguous 2MB blocks
    W2g = nc.declare_dram_parameter("W2g", [8, 128, 4, D], F32, isOutput=False)
    b1t = nc.declare_dram_parameter("b1t", [128, FFT], F32, isOutput=False)
    b2fb = nc.declare_dram_parameter("b2fb", [128, D], F32, isOutput=False)
    a2b = nc.declare_dram_parameter("a2b", [128, D], F32, isOutput=False)
    b2nb = nc.declare_dram_parameter("b2nb", [128, D], F32, isOutput=False)
    VsT = nc.declare_dram_parameter("VsT", [2, 128, D], F32, isOutput=False)
    # all reps write the same output buffer: extra reps add zero host
    # transfer, so reps-diff timing isolates device-side work
    out0 = nc.declare_dram_parameter("out0", [2, 128, D], F32, isOutput=True)
    outs = [out0] * reps

    with tile.TileContext(nc) as tc:
        with (
            tc.tile_pool(name="singles", bufs=1) as singles,
            tc.tile_pool(name="dram", bufs=2, space="DRAM") as dram,
        ):
            ident = singles.tile([128, 128], F32)
            make_identity(nc, ident[:])
            a2_sb = singles.tile([128, D], F32)
            b2n_sb = singles.tile([128, D], F32)
            b1_sb = singles.tile([128, FFT], F32)
            b2f_sb = singles.tile([128, D], F32)
            nc.sync.dma_start(a2_sb[:], a2b[:])
            nc.sync.dma_start(b2n_sb[:], b2nb[:])
            nc.sync.dma_start(b1_sb[:], b1t[:])
            nc.sync.dma_start(b2f_sb[:], b2fb[:])

            for r in range(reps):
                _body(nc, tc, singles, dram, ident, a2_sb, b2n_sb, b1_sb, b2f_sb,
                      Qt, Kt, Vt, wqT, wkT, wvT, Wot, W1g, W2g, VsT, outs[r], r)

    nc.finalize()
    return nc


def _body(nc, tc, singles, dram, ident, a2_sb, b2n_sb, b1_sb, b2f_sb,
          Qt, Kt, Vt, wqT, wkT, wvT, Wot, W1g, W2g, VsT, out, rep):
    import contextlib
    with contextlib.ExitStack() as stack:
        attn = stack.enter_context(tc.tile_pool(name="attn", bufs=1))
        # ---------------- Phase A: projections ----------------
        vq_sb = attn.tile([128, S], F32, name=f"vq_{rep}")   # [2 heads * 64 d, s]
        vk_sb = attn.tile([128, S], F32, name=f"vk_{rep}")
        vvT_sb = attn.tile([128, TT, 2 * (HD + 1)], F32, name=f"vvT_{rep}")

        with (
            tc.tile_pool(name="projw", bufs=1) as projw,
            tc.tile_pool(name="projin", bufs=2) as projin,
            tc.tile_pool(name="projps", bufs=1, space="PSUM") as projps,
            tc.tile_pool(name="trps", bufs=2, space="PSUM") as trps,
        ):
            wq_sb = projw.tile([128, KT, 128], F32)
            wk_sb = projw.tile([128, KT, 128], F32)
            wv_sb = projw.tile([128, KT, 128], F32)
            nc.sync.dma_start(wq_sb[:], wqT[:])
            nc.sync.dma_start(wk_sb[:], wkT[:])
            nc.sync.dma_start(wv_sb[:], wvT[:])

            vv_sb = projw.tile([128, S], F32, name=f"vv_{rep}")
            for (src, wsb, dst) in ((Qt, wq_sb, vq_sb), (Kt, wk_sb, vk_sb), (Vt, wv_sb, vv_sb)):
                ps = projps.tile([128, S], F32, tag="proj_ps", name=f"proj_ps_{rep}")
                for half in range(2):
                    xin = projin.tile([128, 4, S], F32, tag="proj_in",
                                      name=f"proj_in_{rep}_{half}")
                    nc.sync.dma_start(
                        xin[:], src.ap()[half * 4:(half + 1) * 4].rearrange("k p s -> p k s"))
                    for k4 in range(4):
                        k = half * 4 + k4
                        for j in range(NSCH):
                            nc.tensor.matmul(
                                ps[:, j * SCH:(j + 1) * SCH],
                                wsb[:, k, :], xin[:, k4, j * SCH:(j + 1) * SCH],
                                start=(k == 0), stop=(k == KT - 1),
                            )
                for j in range(NSCH):
                    nc.vector.tensor_copy(dst[:, j * SCH:(j + 1) * SCH],
                                          ps[:, j * SCH:(j + 1) * SCH])

            # transpose Vv [(h d), t] -> vvT [t, (d|1)*2] per t_tile, with ones col
            nc.gpsimd.memset(vvT_sb[:], 1.0)  # ones columns come for free
            for t in range(TT):
                pst = trps.tile([128, 128], F32, tag="tr_ps", name=f"trps_{rep}_{t}")
                nc.tensor.transpose(pst[:], vv_sb[:, t * 128:(t + 1) * 128], ident[:])
                nc.vector.tensor_copy(vvT_sb[:, t, 0:HD], pst[:, 0:HD])
                nc.vector.tensor_copy(vvT_sb[:, t, HD + 1:2 * HD + 1], pst[:, HD:2 * HD])

        # ---------------- Phase B: attention per head ----------------
        heads_sb = attn.tile([128, S], F32, name=f"heads_{rep}")  # [(2h x 64d), s] normalized
        send = dram.tile([NCORES, 128, SL], F32, tag="send", name=f"send_{rep}")
        with (
            tc.tile_pool(name="esb", bufs=3) as esb,
            tc.tile_pool(name="scps", bufs=2, space="PSUM") as scps,
            tc.tile_pool(name="avps", bufs=4, space="PSUM") as avps,
            tc.tile_pool(name="avsb", bufs=2) as avsb,
        ):
            for h in range(HPC):
                hp = h * 64        # partition offset of this head in vq/vk
                lo = h * (HD + 1)  # free offset of this head (+ones) in vvT
                ps_h = [avps.tile([128, SCH], F32, tag="av_ps", name=f"av_ps_{rep}_{h}_{j}")
                        for j in range(NSCH)]
                for t in range(TT):
                    e_t = esb.tile([128, S], F32, tag="e", name=f"e_{rep}_{h}_{t}")
                    for half in range(2):
                        ps_s = scps.tile([128, 2 * SCH], F32, tag="sc_ps",
                                         name=f"sc_ps_{rep}_{h}_{t}_{half}")
                        for jj in range(2):
                            j = half * 2 + jj
                            nc.tensor.matmul(
                                ps_s[:, jj * SCH:(jj + 1) * SCH],
                                vk_sb[hp:hp + 64, t * 128:(t + 1) * 128],
                                vq_sb[hp:hp + 64, j * SCH:(j + 1) * SCH],
                                start=True, stop=True,
                            )
                            # E = exp(scores / 8), one bank per ACT op
                            nc.scalar.activation(
                                e_t[:, j * SCH:(j + 1) * SCH],
                                ps_s[:, jj * SCH:(jj + 1) * SCH],
                                AF.Exp, scale=ISCALE)
                    # zero the masked diagonal block (t==s)
                    nc.gpsimd.affine_select(
                        out=e_t[:, t * 128:t * 128 + 128],
                        in_=e_t[:, t * 128:t * 128 + 128],
                        compare_op=mybir.AluOpType.not_equal,
                        fill=0.0, base=0,
                        pattern=[[-1, 128]], channel_multiplier=1,
                    )
                    for j in range(NSCH):
                        nc.tensor.matmul(
                            ps_h[j][0:HD + 1, :],
                            vvT_sb[:, t, lo:lo + HD + 1],
                            e_t[:, j * SCH:(j + 1) * SCH],
                            start=(t == 0), stop=(t == TT - 1),
                        )
                for j in range(NSCH):
                    rec = avsb.tile([1, SCH], F32, tag="av_rec", name=f"rec_{rep}_{h}_{j}")
                    nc.vector.reciprocal(rec[:], ps_h[j][HD:HD + 1, :])
                    rb = avsb.tile([HD, SCH], F32, tag="av_rb", name=f"rb_{rep}_{h}_{j}")
                    nc.gpsimd.partition_broadcast(rb[:], rec[:])
                    nc.vector.tensor_mul(
                        out=heads_sb[hp:hp + 64, j * SCH:(j + 1) * SCH],
                        in0=ps_h[j][0:HD, :], in1=rb[:])
        # one DMA: stage the full head block into the A2A send buffer
        nc.sync.dma_start(send.rearrange("d p s -> p d s"),
                          heads_sb.rearrange("p (d s) -> p d s", d=NCORES))

        # ---------------- Phase C: A2A, Wo, residual + LN1 ----------------
        recv = dram.tile([NCORES, 128, SL], F32, tag="recv", name=f"recv_{rep}")
        if os.environ.get("KERNEL_NO_CC"):
            nc.sync.dma_start(recv[:], send[:])  # timing-only variant: wrong data, right shape
        else:
            nc.gpsimd.collective_compute(
                "AllToAll", mybir.AluOpType.bypass,
                replica_groups=[list(range(NCORES))],
                ins=[send.opt()], outs=[recv.opt()],
            )

        z_sb = attn.tile([128, 2, D], F32, name=f"z_{rep}")      # LN1 out [s_in, s_tile, d]
        xT_sb = attn.tile([128, KT, SL], F32, name=f"xT_{rep}")  # z transposed for FFN rhs
        with (
            tc.tile_pool(name="wophase", bufs=1) as woph,
            tc.tile_pool(name="wops", bufs=4, space="PSUM") as wops,
            tc.tile_pool(name="lnsb", bufs=2) as lnsb,
            tc.tile_pool(name="trps2", bufs=2, space="PSUM") as trps2,
        ):
            recvT = woph.tile([128, NCORES, SL], F32)
            nc.sync.dma_start(recvT[:], recv.rearrange("j p s -> p j s"))
            wo_sb = woph.tile([128, KT, D], F32)
            nc.sync.dma_start(wo_sb[:], Wot[:])
            vs_sb = woph.tile([128, 2, D], F32)
            nc.sync.dma_start(vs_sb[:], VsT.ap().rearrange("st p d -> p st d"))

            for st in range(2):  # two tiles of 128 seq positions
                x_sb = lnsb.tile([128, D], F32, tag="x1", name=f"x1_{rep}_{st}")
                for nchunk in range(2):
                    ps_o = wops.tile([128, SCH], F32, tag="wo_ps", name=f"wo_ps_{rep}_{st}_{nchunk}")
                    for k in range(KT):
                        nc.tensor.matmul(
                            ps_o[:],
                            recvT[:, k, st * 128:(st + 1) * 128],
                            wo_sb[:, k, nchunk * SCH:(nchunk + 1) * SCH],
                            start=(k == 0), stop=(k == KT - 1),
                        )
                    nc.vector.tensor_add(
                        out=x_sb[:, nchunk * SCH:(nchunk + 1) * SCH],
                        in0=ps_o[:],
                        in1=vs_sb[:, st, nchunk * SCH:(nchunk + 1) * SCH],
                    )
                _ln(nc, lnsb, x_sb, z_sb[:, st, :], a2_sb, b2n_sb, f"{rep}_1_{st}")
                for dt in range(KT):
                    pst = trps2.tile([128, 128], F32, tag="tr2_ps", name=f"tr2_{rep}_{st}_{dt}")
                    nc.tensor.transpose(pst[:], z_sb[:, st, dt * 128:(dt + 1) * 128], ident[:])
                    nc.vector.tensor_copy(xT_sb[:, dt, st * 128:(st + 1) * 128], pst[:])

        # ---------------- Phase D: FFN + LN2 + output ----------------
        with (
            tc.tile_pool(name="ffh", bufs=1) as ffhp,
            tc.tile_pool(name="w1s", bufs=2) as w1s,
            tc.tile_pool(name="ffps", bufs=4, space="PSUM") as ffps,
        ):
            ffh_sb = ffhp.tile([128, FFT, SL], F32)
            for g in range(8):
                w1_sb = w1s.tile([128, 4, KT, 128], F32, tag="w1", name=f"w1_{rep}_{g}")
                nc.sync.dma_start(w1_sb[:], W1g[g])
                for mi in range(4):
                    m = g * 4 + mi
                    ps_f = ffps.tile([128, SL], F32, tag="ff_ps", name=f"ff_ps_{rep}_{m}")
                    for k in range(KT):
                        nc.tensor.matmul(
                            ps_f[:], w1_sb[:, mi, k, :], xT_sb[:, k, :],
                            start=(k == 0), stop=(k == KT - 1),
                        )
                    nc.scalar.activation(ffh_sb[:, m, :], ps_f[:], AF.Relu,
                                         bias=b1_sb[:, m:m + 1], scale=1.0)

            with (
                tc.tile_pool(name="w2s", bufs=2) as w2s,
                tc.tile_pool(name="ff2ps", bufs=1, space="PSUM") as ff2ps,
                tc.tile_pool(name="ln2sb", bufs=2) as ln2sb,
            ):
                # swapped FFN2: psum [s, d-chunk] = ffh_tile.T @ W2T_tile
                ps_g = [ff2ps.tile([128, SCH], F32, tag=f"ff2_ps{i}", name=f"ff2_ps_{rep}_{i}")
                        for i in range(4)]
                for g in range(8):
                    w2_sb = w2s.tile([128, 4, D], F32, tag="w2", name=f"w2_{rep}_{g}")
                    nc.sync.dma_start(w2_sb[:], W2g[g])
                    for ki in range(4):
                        k = g * 4 + ki
                        for st in range(2):
                            for dc in range(2):
                                nc.tensor.matmul(
                                    ps_g[st * 2 + dc][:],
                                    ffh_sb[:, k, st * 128:(st + 1) * 128],
                                    w2_sb[:, ki, dc * SCH:(dc + 1) * SCH],
                                    start=(k == 0), stop=(k == FFT - 1),
                                )
                for st in range(2):
                    x2_sb = ln2sb.tile([128, D], F32, tag="x2", name=f"x2_{rep}_{st}")
                    for dc in range(2):
                        nc.vector.tensor_add(
                            out=x2_sb[:, dc * SCH:(dc + 1) * SCH],
                            in0=ps_g[st * 2 + dc][:],
                            in1=z_sb[:, st, dc * SCH:(dc + 1) * SCH],
                        )
                    nc.vector.tensor_add(out=x2_sb[:], in0=x2_sb[:], in1=b2f_sb[:])
                    z2_sb = ln2sb.tile([128, D], F32, tag="z2", name=f"z2_{rep}_{st}")
                    _ln(nc, ln2sb, x2_sb, z2_sb, a2_sb, b2n_sb, f"{rep}_2_{st}")
                    nc.sync.dma_start(out.ap()[st], z2_sb[:])


_NC_CACHE = {}


def _get_nc(reps: int = 1):
    if reps not in _NC_CACHE:
        _NC_CACHE[reps] = build(reps)
    return _NC_CACHE[reps]


def prep_inputs(Q, K, V, wq, wk, wv, Wo, W1, b1, W2, b2, a_2, b_2):
    """Host-side sharding/layout prep. Returns per-core input maps."""
    f32 = np.float32
    Q = np.asarray(Q, f32); K = np.asarray(K, f32); V = np.asarray(V, f32)
    Qt = np.ascontiguousarray(Q.reshape(KT, 128, S))
    Kt = np.ascontiguousarray(K.reshape(KT, 128, S))
    Vt = np.ascontiguousarray(V.reshape(KT, 128, S))
    Wot = np.ascontiguousarray(np.asarray(Wo, f32).reshape(KT, 128, D).transpose(1, 0, 2))
    # W1g: [8, 128(ki), 4(m-sub), 8(kt), 128(m)]
    W1g = np.ascontiguousarray(
        np.asarray(W1, f32).reshape(8, 4, 128, KT, 128)  # [g, msub, m, kt, ki]
        .transpose(0, 4, 1, 3, 2))                        # -> [g, ki, msub, kt, m]
    # W2g: [8, 128(ki over f), 4(kt-sub over f), 1024(d)]
    W2g = np.ascontiguousarray(
        np.asarray(W2, f32).T.reshape(8, 4, 128, D).transpose(0, 2, 1, 3))
    b1t = np.ascontiguousarray(np.asarray(b1, f32).reshape(FFT, 128).T)
    b2fb = np.ascontiguousarray(np.broadcast_to(np.asarray(b2, f32), (128, D)))
    a2b = np.ascontiguousarray(np.broadcast_to(np.asarray(a_2, f32), (128, D)))
    b2nb = np.ascontiguousarray(np.broadcast_to(np.asarray(b_2, f32), (128, D)))

    wq = np.asarray(wq, f32); wk = np.asarray(wk, f32); wv = np.asarray(wv, f32)
    in_maps = []
    for c in range(NCORES):
        def _wT(w):
            wc = w[c * HPC:(c + 1) * HPC].reshape(128, D)  # [m, k]
            return np.ascontiguousarray(wc.reshape(128, KT, 128).transpose(2, 1, 0))
        VsT = np.ascontiguousarray(V[:, c * SL:(c + 1) * SL].T.reshape(2, 128, D))
        in_maps.append({
            "Qt": Qt, "Kt": Kt, "Vt": Vt,
            "wqT": _wT(wq), "wkT": _wT(wk), "wvT": _wT(wv),
            "Wot": Wot, "W1g": W1g, "W2g": W2g,
            "b1t": b1t, "b2fb": b2fb, "a2b": a2b, "b2nb": b2nb,
            "VsT": VsT,
        })
    return in_maps


def run(in_maps, reps: int = 1):
    nc = _get_nc(reps)
    return run_bass_kernel_spmd(nc, in_maps, list(range(NCORES)))


def assemble(results, rep=0):
    """[2,128,1024] per core -> full [1024, 2048] output."""
    z2 = np.concatenate(
        [results[c][f"out{rep}"].reshape(2 * 128, D) for c in range(NCORES)], axis=0)
    return np.ascontiguousarray(z2.T)


def kernel(Q, K, V, wq, wk, wv, Wo, W1, b1, W2, b2, a_2, b_2):
    in_maps = prep_inputs(Q, K, V, wq, wk, wv, Wo, W1, b1, W2, b2, a_2, b_2)
    res = run(in_maps, reps=1).results
    return assemble(res)



# revision 7
# speedup vs baseline: 683.9523x; 683.9523x over previous
"""Trainium2 Bass kernel for nn_EncoderLayer (D=1024, H=16, S=2048, FF=4096), 8-core SPMD.

Strategy: head-parallel attention (2 heads/core), one 1MB AllToAll to switch to
sequence-parallel (256 positions/core) for the output projection, norms and FFN.
No all-reduce needed anywhere. v2: instruction/DMA-count minimized.
"""
import math
import os

import numpy as np

import concourse.bass as bass
import concourse.mybir as mybir
import concourse.tile as tile
from concourse import bacc
from concourse.bass_utils import run_bass_kernel_spmd
from concourse.masks import make_identity

F32 = mybir.dt.float32
F32R = mybir.dt.float32r
BF16 = mybir.dt.bfloat16
AF = mybir.ActivationFunctionType

D = 1024
H = 16
HD = 64
S = 2048
FF = 4096
EPS = 1e-3
NCORES = 8
SL = S // NCORES          # 256 sequence positions per core after A2A
HPC = H // NCORES         # 2 heads per core
KT = D // 128             # 8 k-tiles over the model dim
TT = S // 128             # 16 t-tiles over sequence
SCH = 512                 # matmul moving-operand chunk (fp32 max)
NSCH = S // SCH           # 4 s-chunks
FFT = FF // 128           # 32 hidden tiles
UNBIAS = float(D) / float(D - 1)  # torch std uses ddof=1
ISCALE = 1.0 / math.sqrt(HD)


def _mm(nc, out, lhsT, rhs, **kw):
    """Matmul on float32r operands: 4x PE throughput vs fp32 (moving dim >= 256)."""
    nc.tensor.matmul(out, lhsT, rhs, **kw)


def _ln(nc, pools, x_sb, z_sb, a2_sb, b2n_sb, tag):
    """LayerNorm over free axis (1024) of x_sb [128, 1024] -> z_sb [128, 1024].

    Matches reference: (x - mu) / (std_ddof1 + eps) * a2 + b2.
    """
    s1 = pools.tile([128, 1], F32, tag=f"ln_s1", name=f"ln_s1_{tag}")
    nc.vector.reduce_sum(out=s1[:], in_=x_sb[:], axis=mybir.AxisListType.X)
    mu = pools.tile([128, 1], F32, tag=f"ln_mu", name=f"ln_mu_{tag}")
    nc.scalar.mul(mu[:], s1[:], 1.0 / D)
    xc = pools.tile([128, D], F32, tag=f"ln_xc", name=f"ln_xc_{tag}")
    nc.vector.tensor_scalar(out=xc[:], in0=x_sb[:], scalar1=mu[:], scalar2=None,
                            op0=mybir.AluOpType.subtract)
    sq = pools.tile([128, D], F32, tag=f"ln_sq", name=f"ln_sq_{tag}")
    nc.vector.tensor_mul(out=sq[:], in0=xc[:], in1=xc[:])
    s2 = pools.tile([128, 1], F32, tag=f"ln_s2", name=f"ln_s2_{tag}")
    nc.vector.reduce_sum(out=s2[:], in_=sq[:], axis=mybir.AxisListType.X)
    sig = pools.tile([128, 1], F32, tag=f"ln_sig", name=f"ln_sig_{tag}")
    # sigma = sqrt(ssq / (D-1)); then += eps; then reciprocal
    nc.scalar.activation(sig[:], s2[:], AF.Sqrt, scale=1.0 / (D - 1))
    nc.vector.tensor_scalar_add(sig[:], sig[:], EPS)
    rec = pools.tile([128, 1], F32, tag=f"ln_rec", name=f"ln_rec_{tag}")
    nc.vector.reciprocal(rec[:], sig[:])
    nc.vector.tensor_scalar_mul(z_sb[:], xc[:], rec[:])
    nc.vector.tensor_mul(out=z_sb[:], in0=z_sb[:], in1=a2_sb[:])
    nc.vector.tensor_add(out=z_sb[:], in0=z_sb[:], in1=b2n_sb[:])


def build(reps: int = 1):
    nc = bacc.Bacc("TRN2", target_bir_lowering=False, debug=False, num_devices=NCORES)

    # ---- DRAM parameters (per-core shards prepared on host) ----
    Qt = nc.declare_dram_parameter("Qt", [KT, 128, S], F32R, isOutput=False)
    Kt = nc.declare_dram_parameter("Kt", [KT, 128, S], F32R, isOutput=False)
    Vt = nc.declare_dram_parameter("Vt", [KT, 128, S], BF16, isOutput=False)
    wqT = nc.declare_dram_parameter("wqT", [128, KT, 128], F32R, isOutput=False)
    wkT = nc.declare_dram_parameter("wkT", [128, KT, 128], F32R, isOutput=False)
    wvT = nc.declare_dram_parameter("wvT", [128, KT, 128], BF16, isOutput=False)
    Wot = nc.declare_dram_parameter("Wot", [128, KT, D], BF16, isOutput=False)
    # W1g[g] : [128, 4, 8, 128]  (ki, m-sub, kt, m)  contiguous 2MB blocks
    W1g = nc.declare_dram_parameter("W1g", [8, 128, 4, KT, 128], BF16, isOutput=False)
    # W2g[g] : [128, 4, 1024]  (ki(f), kt-sub(f), d) contiguous 2MB blocks
    W2g = nc.declare_dram_parameter("W2g", [8, 128, 4, D], BF16, isOutput=False)
    b1t = nc.declare_dram_parameter("b1t", [128, FFT], F32, isOutput=False)
    b2fb = nc.declare_dram_parameter("b2fb", [128, D], F32, isOutput=False)
    a2b = nc.declare_dram_parameter("a2b", [128, D], F32, isOutput=False)
    b2nb = nc.declare_dram_parameter("b2nb", [128, D], F32, isOutput=False)
    VsT = nc.declare_dram_parameter("VsT", [2, 128, D], F32, isOutput=False)
    # all reps write the same output buffer: extra reps add zero host
    # transfer, so reps-diff timing isolates device-side work
    out0 = nc.declare_dram_parameter("out0", [2, 128, D], F32, isOutput=True)
    outs = [out0] * reps

    with tile.TileContext(nc) as tc:
        with (
            tc.tile_pool(name="singles", bufs=1) as singles,
            tc.tile_pool(name="dram", bufs=2, space="DRAM") as dram,
        ):
            ident = singles.tile([128, 128], F32)
            make_identity(nc, ident[:])
            a2_sb = singles.tile([128, D], F32)
            b2n_sb = singles.tile([128, D], F32)
            b1_sb = singles.tile([128, FFT], F32)
            b2f_sb = singles.tile([128, D], F32)
            nc.sync.dma_start(a2_sb[:], a2b[:])
            nc.sync.dma_start(b2n_sb[:], b2nb[:])
            nc.sync.dma_start(b1_sb[:], b1t[:])
            nc.sync.dma_start(b2f_sb[:], b2fb[:])

            for r in range(reps):
                _body(nc, tc, singles, dram, ident, a2_sb, b2n_sb, b1_sb, b2f_sb,
                      Qt, Kt, Vt, wqT, wkT, wvT, Wot, W1g, W2g, VsT, outs[r], r)

    nc.finalize()
    return nc


def _body(nc, tc, singles, dram, ident, a2_sb, b2n_sb, b1_sb, b2f_sb,
          Qt, Kt, Vt, wqT, wkT, wvT, Wot, W1g, W2g, VsT, out, rep):
    import contextlib
    with contextlib.ExitStack() as stack:
        attn = stack.enter_context(tc.tile_pool(name="attn", bufs=1))
        # ---------------- Phase A: projections ----------------
        vq_sb = attn.tile([128, S], F32R, name=f"vq_{rep}")   # [2 heads * 64 d, s]
        vk_sb = attn.tile([128, S], F32R, name=f"vk_{rep}")
        vvT_sb = attn.tile([128, TT, 2 * (HD + 1)], BF16, name=f"vvT_{rep}")

        with (
            tc.tile_pool(name="projw", bufs=1) as projw,
            tc.tile_pool(name="projin", bufs=2) as projin,
            tc.tile_pool(name="projps", bufs=1, space="PSUM") as projps,
            tc.tile_pool(name="trps", bufs=2, space="PSUM") as trps,
        ):
            wq_sb = projw.tile([128, KT, 128], F32R)
            wk_sb = projw.tile([128, KT, 128], F32R)
            wv_sb = projw.tile([128, KT, 128], BF16)
            nc.sync.dma_start(wq_sb[:], wqT[:])
            nc.sync.dma_start(wk_sb[:], wkT[:])
            nc.sync.dma_start(wv_sb[:], wvT[:])

            vv_sb = projw.tile([128, S], F32, name=f"vv_{rep}")
            for (src, wsb, dst, xdt) in ((Qt, wq_sb, vq_sb, F32R), (Kt, wk_sb, vk_sb, F32R),
                                         (Vt, wv_sb, vv_sb, BF16)):
                ps = projps.tile([128, S], F32, tag="proj_ps", name=f"proj_ps_{rep}")
                for half in range(2):
                    xin = projin.tile([128, 4, S], xdt, tag="proj_in",
                                      name=f"proj_in_{rep}_{half}")
                    nc.sync.dma_start(
                        xin[:], src.ap()[half * 4:(half + 1) * 4].rearrange("k p s -> p k s"))
                    for k4 in range(4):
                        k = half * 4 + k4
                        for j in range(NSCH):
                            _mm(nc,
                                ps[:, j * SCH:(j + 1) * SCH],
                                wsb[:, k, :], xin[:, k4, j * SCH:(j + 1) * SCH],
                                start=(k == 0), stop=(k == KT - 1),
                            )
                for j in range(NSCH):
                    nc.vector.tensor_copy(dst[:, j * SCH:(j + 1) * SCH],
                                          ps[:, j * SCH:(j + 1) * SCH])

            # transpose Vv [(h d), t] -> vvT [t, (d|1)*2] per t_tile, with ones col
            # (memset can't write f32r: fill an f32 twin, convert via one copy)
            ones_f32 = projw.tile([128, TT, 2 * (HD + 1)], F32, name=f"ones_{rep}")
            nc.gpsimd.memset(ones_f32[:], 1.0)
            nc.vector.tensor_copy(vvT_sb[:], ones_f32[:])  # ones columns come for free
            for t in range(TT):
                pst = trps.tile([128, 128], F32, tag="tr_ps", name=f"trps_{rep}_{t}")
                nc.tensor.transpose(pst[:], vv_sb[:, t * 128:(t + 1) * 128], ident[:])
                nc.vector.tensor_copy(vvT_sb[:, t, 0:HD], pst[:, 0:HD])
                nc.vector.tensor_copy(vvT_sb[:, t, HD + 1:2 * HD + 1], pst[:, HD:2 * HD])

        # ---------------- Phase B: attention per head ----------------
        heads_sb = attn.tile([128, S], BF16, name=f"heads_{rep}")  # [(2h x 64d), s] normalized
        send = dram.tile([NCORES, 128, SL], BF16, tag="send", name=f"send_{rep}")
        with (
            tc.tile_pool(name="esb", bufs=3) as esb,
            tc.tile_pool(name="scps", bufs=2, space="PSUM") as scps,
            tc.tile_pool(name="avps", bufs=4, space="PSUM") as avps,
            tc.tile_pool(name="avsb", bufs=2) as avsb,
        ):
            for h in range(HPC):
                hp = h * 64        # partition offset of this head in vq/vk
                lo = h * (HD + 1)  # free offset of this head (+ones) in vvT
                ps_h = [avps.tile([128, SCH], F32, tag="av_ps", name=f"av_ps_{rep}_{h}_{j}")
                        for j in range(NSCH)]
                for t in range(TT):
                    e_t = esb.tile([128, S], BF16, tag="e", name=f"e_{rep}_{h}_{t}")
                    for half in range(2):
                        ps_s = scps.tile([128, 2 * SCH], F32, tag="sc_ps",
                                         name=f"sc_ps_{rep}_{h}_{t}_{half}")
                        for jj in range(2):
                            j = half * 2 + jj
                            _mm(nc,
                                ps_s[:, jj * SCH:(jj + 1) * SCH],
                                vk_sb[hp:hp + 64, t * 128:(t + 1) * 128],
                                vq_sb[hp:hp + 64, j * SCH:(j + 1) * SCH],
                                start=True, stop=True,
                            )
                            # E = exp(scores / 8), one bank per ACT op
                            nc.scalar.activation(
                                e_t[:, j * SCH:(j + 1) * SCH],
                                ps_s[:, jj * SCH:(jj + 1) * SCH],
                                AF.Exp, scale=ISCALE)
                    # zero the masked diagonal block (t==s)
                    nc.gpsimd.affine_select(
                        out=e_t[:, t * 128:t * 128 + 128],
                        in_=e_t[:, t * 128:t * 128 + 128],
                        compare_op=mybir.AluOpType.not_equal,
                        fill=0.0, base=0,
                        pattern=[[-1, 128]], channel_multiplier=1,
                    )
                    for j in range(NSCH):
                        _mm(nc,
                            ps_h[j][0:HD + 1, :],
                            vvT_sb[:, t, lo:lo + HD + 1],
                            e_t[:, j * SCH:(j + 1) * SCH],
                            start=(t == 0), stop=(t == TT - 1),
                        )
                for j in range(NSCH):
                    rec = avsb.tile([1, SCH], F32, tag="av_rec", name=f"rec_{rep}_{h}_{j}")
                    nc.vector.reciprocal(rec[:], ps_h[j][HD:HD + 1, :])
                    rb = avsb.tile([HD, SCH], F32, tag="av_rb", name=f"rb_{rep}_{h}_{j}")
                    nc.gpsimd.partition_broadcast(rb[:], rec[:])
                    nc.vector.tensor_mul(
                        out=heads_sb[hp:hp + 64, j * SCH:(j + 1) * SCH],
                        in0=ps_h[j][0:HD, :], in1=rb[:])
        # one DMA: stage the full head block into the A2A send buffer
        nc.sync.dma_start(send.rearrange("d p s -> p d s"),
                          heads_sb.rearrange("p (d s) -> p d s", d=NCORES))

        # ---------------- Phase C: A2A, Wo, residual + LN1 ----------------
        recv = dram.tile([NCORES, 128, SL], BF16, tag="recv", name=f"recv_{rep}")
        if os.environ.get("KERNEL_NO_CC"):
            nc.sync.dma_start(recv[:], send[:])  # timing-only variant: wrong data, right shape
        else:
            nc.gpsimd.collective_compute(
                "AllToAll", mybir.AluOpType.bypass,
                replica_groups=[list(range(NCORES))],
                ins=[send.opt()], outs=[recv.opt()],
            )

        z_sb = attn.tile([128, 2, D], F32, name=f"z_{rep}")      # LN1 out [s_in, s_tile, d]
        xT_sb = attn.tile([128, KT, SL], BF16, name=f"xT_{rep}")  # z transposed for FFN rhs
        with (
            tc.tile_pool(name="wophase", bufs=1) as woph,
            tc.tile_pool(name="wops", bufs=4, space="PSUM") as wops,
            tc.tile_pool(name="lnsb", bufs=2) as lnsb,
            tc.tile_pool(name="trps2", bufs=2, space="PSUM") as trps2,
        ):
            recvT = woph.tile([128, NCORES, SL], BF16)
            nc.sync.dma_start(recvT[:], recv.rearrange("j p s -> p j s"))
            wo_sb = woph.tile([128, KT, D], BF16)
            nc.sync.dma_start(wo_sb[:], Wot[:])
            vs_sb = woph.tile([128, 2, D], F32)
            nc.sync.dma_start(vs_sb[:], VsT.ap().rearrange("st p d -> p st d"))

            for st in range(2):  # two tiles of 128 seq positions
                x_sb = lnsb.tile([128, D], F32, tag="x1", name=f"x1_{rep}_{st}")
                for nchunk in range(2):
                    ps_o = wops.tile([128, SCH], F32, tag="wo_ps", name=f"wo_ps_{rep}_{st}_{nchunk}")
                    for k in range(KT):
                        _mm(nc,
                            ps_o[:],
                            recvT[:, k, st * 128:(st + 1) * 128],
                            wo_sb[:, k, nchunk * SCH:(nchunk + 1) * SCH],
                            start=(k == 0), stop=(k == KT - 1),
                        )
                    nc.vector.tensor_add(
                        out=x_sb[:, nchunk * SCH:(nchunk + 1) * SCH],
                        in0=ps_o[:],
                        in1=vs_sb[:, st, nchunk * SCH:(nchunk + 1) * SCH],
                    )
                _ln(nc, lnsb, x_sb, z_sb[:, st, :], a2_sb, b2n_sb, f"{rep}_1_{st}")
                for dt in range(KT):
                    pst = trps2.tile([128, 128], F32, tag="tr2_ps", name=f"tr2_{rep}_{st}_{dt}")
                    nc.tensor.transpose(pst[:], z_sb[:, st, dt * 128:(dt + 1) * 128], ident[:])
                    nc.vector.tensor_copy(xT_sb[:, dt, st * 128:(st + 1) * 128], pst[:])

        # ---------------- Phase D: FFN + LN2 + output ----------------
        with (
            tc.tile_pool(name="ffh", bufs=1) as ffhp,
            tc.tile_pool(name="w1s", bufs=2) as w1s,
            tc.tile_pool(name="ffps", bufs=4, space="PSUM") as ffps,
        ):
            ffh_sb = ffhp.tile([128, FFT, SL], BF16)
            for g in range(8):
                w1_sb = w1s.tile([128, 4, KT, 128], BF16, tag="w1", name=f"w1_{rep}_{g}")
                nc.sync.dma_start(w1_sb[:], W1g[g])
                for mi in range(4):
                    m = g * 4 + mi
                    ps_f = ffps.tile([128, SL], F32, tag="ff_ps", name=f"ff_ps_{rep}_{m}")
                    for k in range(KT):
                        _mm(nc,
                            ps_f[:], w1_sb[:, mi, k, :], xT_sb[:, k, :],
                            start=(k == 0), stop=(k == KT - 1),
                        )
                    nc.scalar.activation(ffh_sb[:, m, :], ps_f[:], AF.Relu,
                                         bias=b1_sb[:, m:m + 1], scale=1.0)

            with (
                tc.tile_pool(name="w2s", bufs=2) as w2s,
                tc.tile_pool(name="ff2ps", bufs=1, space="PSUM") as ff2ps,
                tc.tile_pool(name="ln2sb", bufs=2) as ln2sb,
            ):
                # swapped FFN2: psum [s, d-chunk] = ffh_tile.T @ W2T_tile
                ps_g = [ff2ps.tile([128, SCH], F32, tag=f"ff2_ps{i}", name=f"ff2_ps_{rep}_{i}")
                        for i in range(4)]
                for g in range(8):
                    w2_sb = w2s.tile([128, 4, D], BF16, tag="w2", name=f"w2_{rep}_{g}")
                    nc.sync.dma_start(w2_sb[:], W2g[g])
                    for ki in range(4):
                        k = g * 4 + ki
                        for st in range(2):
                            for dc in range(2):
                                _mm(nc,
                                    ps_g[st * 2 + dc][:],
                                    ffh_sb[:, k, st * 128:(st + 1) * 128],
                                    w2_sb[:, ki, dc * SCH:(dc + 1) * SCH],
                                    start=(k == 0), stop=(k == FFT - 1),
                                )
                for st in range(2):
                    x2_sb = ln2sb.tile([128, D], F32, tag="x2", name=f"x2_{rep}_{st}")
                    for dc in range(2):
                        nc.vector.tensor_add(
                            out=x2_sb[:, dc * SCH:(dc + 1) * SCH],
                            in0=ps_g[st * 2 + dc][:],
                            in1=z_sb[:, st, dc * SCH:(dc + 1) * SCH],
                        )
                    nc.vector.tensor_add(out=x2_sb[:], in0=x2_sb[:], in1=b2f_sb[:])
                    z2_sb = ln2sb.tile([128, D], F32, tag="z2", name=f"z2_{rep}_{st}")
                    _ln(nc, ln2sb, x2_sb, z2_sb, a2_sb, b2n_sb, f"{rep}_2_{st}")
                    nc.sync.dma_start(out.ap()[st], z2_sb[:])


_NC_CACHE = {}


def _get_nc(reps: int = 1):
    if reps not in _NC_CACHE:
        _NC_CACHE[reps] = build(reps)
    return _NC_CACHE[reps]


def prep_inputs(Q, K, V, wq, wk, wv, Wo, W1, b1, W2, b2, a_2, b_2):
    """Host-side sharding/layout prep. Returns per-core input maps."""
    import ml_dtypes
    f32 = np.float32
    bf16 = ml_dtypes.bfloat16
    Q = np.asarray(Q, f32); K = np.asarray(K, f32); V = np.asarray(V, f32)
    Qt = np.ascontiguousarray(Q.reshape(KT, 128, S))
    Kt = np.ascontiguousarray(K.reshape(KT, 128, S))
    Vt = np.ascontiguousarray(V.reshape(KT, 128, S).astype(bf16))
    Wot = np.ascontiguousarray(np.asarray(Wo, f32).reshape(KT, 128, D).transpose(1, 0, 2).astype(bf16))
    # W1g: [8, 128(ki), 4(m-sub), 8(kt), 128(m)]
    W1g = np.ascontiguousarray(
        np.asarray(W1, f32).reshape(8, 4, 128, KT, 128)  # [g, msub, m, kt, ki]
        .transpose(0, 4, 1, 3, 2).astype(bf16))           # -> [g, ki, msub, kt, m]
    # W2g: [8, 128(ki over f), 4(kt-sub over f), 1024(d)]
    W2g = np.ascontiguousarray(
        np.asarray(W2, f32).T.reshape(8, 4, 128, D).transpose(0, 2, 1, 3).astype(bf16))
    b1t = np.ascontiguousarray(np.asarray(b1, f32).reshape(FFT, 128).T)
    b2fb = np.ascontiguousarray(np.broadcast_to(np.asarray(b2, f32), (128, D)))
    a2b = np.ascontiguousarray(np.broadcast_to(np.asarray(a_2, f32), (128, D)))
    b2nb = np.ascontiguousarray(np.broadcast_to(np.asarray(b_2, f32), (128, D)))

    wq = np.asarray(wq, f32); wk = np.asarray(wk, f32); wv = np.asarray(wv, f32)
    in_maps = []
    for c in range(NCORES):
        def _wT(w):
            wc = w[c * HPC:(c + 1) * HPC].reshape(128, D)  # [m, k]
            return np.ascontiguousarray(wc.reshape(128, KT, 128).transpose(2, 1, 0))
        VsT = np.ascontiguousarray(V[:, c * SL:(c + 1) * SL].T.reshape(2, 128, D))
        in_maps.append({
            "Qt": Qt, "Kt": Kt, "Vt": Vt,
            "wqT": _wT(wq), "wkT": _wT(wk), "wvT": _wT(wv).astype(bf16),
            "Wot": Wot, "W1g": W1g, "W2g": W2g,
            "b1t": b1t, "b2fb": b2fb, "a2b": a2b, "b2nb": b2nb,
            "VsT": VsT,
        })
    return in_maps


def run(in_maps, reps: int = 1):
    nc = _get_nc(reps)
    return run_bass_kernel_spmd(nc, in_maps, list(range(NCORES)))


def assemble(results, rep=0):
    """[2,128,1024] per core -> full [1024, 2048] output."""
    z2 = np.concatenate(
        [results[c][f"out{rep}"].reshape(2 * 128, D) for c in range(NCORES)], axis=0)
    return np.ascontiguousarray(z2.T)


def kernel(Q, K, V, wq, wk, wv, Wo, W1, b1, W2, b2, a_2, b_2):
    in_maps = prep_inputs(Q, K, V, wq, wk, wv, Wo, W1, b1, W2, b2, a_2, b_2)
    res = run(in_maps, reps=1).results
    return assemble(res)

